# revision 1
# baseline (speedup 1.0000x reference)
"""Trainium2 Bass kernel for the DependencyParseModel problem.

Pipeline (replicated biLSTM, pairwise scoring sharded over 8 cores):
  1. host: embedding gather, weight permute/pad into PE-friendly layouts
  2. device: XW = Wih @ x_aug  (dense matmuls), 256-step LSTM scans with
     weights-stationary matvecs (bf16 FWL), both directions interleaved
  3. device: pairwise-MLP scoring for this core's 32 head rows
  4. host: assemble [256,256], add b2, zero diagonal

Self-contained: hardcodes all shapes; no sibling imports.
"""

import os
import numpy as np
import ml_dtypes

T = 256
H = 400            # LSTM hidden
HP = 512           # padded hidden
G = 2048           # padded gate dim (4 gates x 512)
D1P = 1024         # padded layer-1 input dim (2 x HP)
MLP = 400
NCORES = 8
RPC = T // NCORES  # rows per core (head-word rows)

BF16 = ml_dtypes.bfloat16

# gate blocks in OUR layout order [f, i, g~, o]; source ranges in torch order
# (f first / o last so the pointwise chain pipelines with the matvec groups)
_GATE_SRC = [(400, 800), (0, 400), (800, 1200), (1200, 1600)]


def _permute_pad_gate_rows(W):
    """[1600, K] -> [2048, K]: torch gate order i,f,g,o -> blocks [i,f,o,g~], each padded to 512."""
    out = np.zeros((G, W.shape[1]), np.float32)
    for b, (s, e) in enumerate(_GATE_SRC):
        out[b * 512: b * 512 + (e - s)] = W[s:e]
    return out


def _permute_pad_gate_vec(v):
    out = np.zeros(G, np.float32)
    for b, (s, e) in enumerate(_GATE_SRC):
        out[b * 512: b * 512 + (e - s)] = v[s:e]
    return out


def _pad_cols(W, K):
    """[R, k] -> [R, K] zero-padded."""
    out = np.zeros((W.shape[0], K), np.float32)
    out[:, : W.shape[1]] = W
    return out


def _h_tile(v):
    """[400] -> [128, 4] tile, elem d -> (d % 128, d // 128)."""
    out = np.zeros(HP, np.float32)
    out[:H] = v
    return np.ascontiguousarray(out.reshape(4, 128).T)


_PROG_CACHE = {}


def _get_program(n_steps=T):
    key = ("prog", n_steps)
    if key in _PROG_CACHE:
        return _PROG_CACHE[key]

    import concourse.bass as bass
    import concourse.mybir as mybir
    import concourse.tile as tile
    from concourse import bacc
    from concourse import hw_specs as _hs

    # Price the PE decode at the real ~27ns/matvec FWL pace for any
    # consumer of hw_specs (CoreSim etc).  Measured no effect on the tile
    # scheduler itself (its Rust state does not re-read this), but kept so
    # simulated timings resemble hardware.
    _hs.TRN2Spec.EXPECTED_HWDECODE_OVERHEAD_NS[mybir.EngineType.PE] = 27.0

    F32 = mybir.dt.float32
    BF = mybir.dt.bfloat16
    AF = mybir.ActivationFunctionType
    ALU = mybir.AluOpType

    nc = bacc.Bacc("TRN2", target_bir_lowering=False, debug=False,
                   enable_asserts=False, num_devices=NCORES)

    # ---- DRAM I/O ----
    d_xT = nc.dram_tensor("xT", [HP, T], BF, kind="ExternalInput").ap()
    d_wihT0 = [nc.dram_tensor(f"wihT0{d}", [HP, G], BF, kind="ExternalInput").ap() for d in "fb"]
    d_wihT1 = [nc.dram_tensor(f"wihT1{d}", [D1P, G], BF, kind="ExternalInput").ap() for d in "fb"]
    d_whhT = [[nc.dram_tensor(f"whhT{l}{d}", [HP, G], BF, kind="ExternalInput").ap() for d in "fb"]
              for l in (0, 1)]
    d_h0 = [[nc.dram_tensor(f"h0_{l}{d}", [128, 4], BF, kind="ExternalInput").ap() for d in "fb"]
            for l in (0, 1)]
    d_c0 = [[nc.dram_tensor(f"c0_{l}{d}", [128, 4], F32, kind="ExternalInput").ap() for d in "fb"]
            for l in (0, 1)]
    d_w1aT = nc.dram_tensor("w1aT", [D1P, MLP], BF, kind="ExternalInput").ap()
    d_w1bT = nc.dram_tensor("w1bT", [D1P, MLP], BF, kind="ExternalInput").ap()
    d_skT = nc.dram_tensor("skT", [T, RPC], BF, kind="ExternalInput").ap()
    d_w2rep = nc.dram_tensor("w2rep", [128, MLP], BF, kind="ExternalInput").ap()
    d_ident = nc.dram_tensor("ident", [128, 128], BF, kind="ExternalInput").ap()
    d_out = nc.dram_tensor("scores_t", [T, RPC], F32, kind="ExternalOutput").ap()

    with tile.TileContext(nc) as tc:
        from contextlib import ExitStack
        with ExitStack() as ctx:
            const = ctx.enter_context(tc.tile_pool(name="const", bufs=1))
            state = ctx.enter_context(tc.tile_pool(name="state", bufs=1))
            whhp = ctx.enter_context(tc.tile_pool(name="whhp", bufs=1))

            # --- constants / initial state in SBUF ---
            # DMA order matters: the sync queue drains in emission order, so
            # everything the XW0 phase + L0 scan needs comes first; the
            # layer-1 / scoring weights are deferred below (they overlap with
            # the L0 scan).
            xT_sb = []
            for kc in range(4):
                xt = const.tile([128, T], BF, name=f"xT{kc}")
                nc.sync.dma_start(xt, d_xT[128 * kc:128 * (kc + 1), :])
                xT_sb.append(xt)
            ident_sb = const.tile([128, 128], BF, name="ident")
            nc.sync.dma_start(ident_sb, d_ident)
            h0_sb = [[None, None], [None, None]]
            c_sb = [[None, None], [None, None]]
            for l in (0, 1):
                for d in (0, 1):
                    t0 = const.tile([128, 4], BF, name=f"h0sb{l}{d}")
                    nc.sync.dma_start(t0, d_h0[l][d])
                    h0_sb[l][d] = t0
                    t1 = state.tile([128, 4], F32, name=f"csb{l}{d}")
                    nc.sync.dma_start(t1, d_c0[l][d])
                    c_sb[l][d] = t1
            whh_sb = [[None, None], [None, None]]
            for l in (0,):
                for d in (0, 1):
                    chunks = []
                    for kc in range(4):
                        w = whhp.tile([128, G], BF, name=f"whh{l}{d}{kc}")
                        nc.sync.dma_start(w, d_whhT[l][d][128 * kc:128 * (kc + 1), :])
                        chunks.append(w)
                    whh_sb[l][d] = chunks
            ones_sb = const.tile([1, 128], BF, name="ones")
            nc.vector.memset(ones_sb, 1.0)

            def load_deferred_consts():
                """L1-scan + scoring weights; issued after XW0 so they ride
                the DMA queue during the L0 scan."""
                for d in (0, 1):
                    chunks = []
                    for kc in range(4):
                        w = whhp.tile([128, G], BF, name=f"whh1{d}{kc}")
                        nc.sync.dma_start(w, d_whhT[1][d][128 * kc:128 * (kc + 1), :])
                        chunks.append(w)
                    whh_sb[1][d] = chunks
                for kc in range(8):
                    wa = const.tile([128, MLP], BF, name=f"w1aT{kc}")
                    nc.sync.dma_start(wa, d_w1aT[128 * kc:128 * (kc + 1), :])
                    w1aT_sb.append(wa)
                    wb = const.tile([128, MLP], BF, name=f"w1bT{kc}")
                    nc.sync.dma_start(wb, d_w1bT[128 * kc:128 * (kc + 1), :])
                    w1bT_sb.append(wb)
                for kc in range(2):
                    sk = const.tile([128, RPC], BF, name=f"skT{kc}")
                    nc.sync.dma_start(sk, d_skT[128 * kc:128 * (kc + 1), :])
                    skT_sb.append(sk)
                w2 = const.tile([128, MLP], BF, name="w2rep")
                nc.sync.dma_start(w2, d_w2rep)
                return w2

            w1aT_sb, w1bT_sb, skT_sb = [], [], []

            # persistent per-(layer,dir) h history [128, 4*T] bf16, col 4t+c = h_t[128c+p]
            hh_sb = [[state.tile([128, 4 * T], BF, name=f"hh{l}{d}") for d in (0, 1)]
                     for l in (0, 1)]
            # XW^T buffers, reused across layers: [128, 16*T] bf16, col m*T + t
            # (bf16 so the per-step XW add can ride the PE as an identity-matmul)
            xwt_sb = [state.tile([128, 16 * T], BF, name=f"xwt{d}") for d in (0, 1)]

            def xwt_phase(layer, preloaded=None):
                """xwt_sb[d] <- Wih[layer][d] @ x_aug (all timesteps)."""
                K = 4 if layer == 0 else 8
                d_wih = d_wihT0 if layer == 0 else d_wihT1
                with tc.tile_pool(name=f"wihp{layer}", bufs=1) as wp, \
                     tc.tile_pool(name=f"xwps{layer}", bufs=8, space="PSUM") as pp:
                    for d in (0, 1):
                        if preloaded is not None:
                            wih_sb = preloaded[d]
                        else:
                            wih_sb = []
                            for kc in range(K):
                                w = wp.tile([128, G], BF, name=f"wih{layer}{d}{kc}",
                                            tag=f"wih{kc}")
                                nc.sync.dma_start(w, d_wih[d][128 * kc:128 * (kc + 1), :])
                                wih_sb.append(w)
                        if layer == 0:
                            rhs = xT_sb
                        else:
                            rhs = []
                            for kc in range(K):
                                hhr = hh_sb[0][kc // 4][:].rearrange(
                                    "p (t c) -> p c t", c=4)
                                rhs.append(hhr[:, kc % 4, :])
                        for m in range(16):
                            ps = pp.tile([128, T], F32, name=f"xwps{layer}{d}{m}",
                                         tag="xwps")
                            for kc in range(K):
                                nc.tensor.matmul(
                                    ps, wih_sb[kc][:, 128 * m:128 * (m + 1)], rhs[kc],
                                    start=(kc == 0), stop=(kc == K - 1))
                            # alternate psum->sbuf copies between DVE and the
                            # idle ACT engine so they don't serialize the phase
                            if m % 2 == 0:
                                nc.vector.tensor_copy(
                                    xwt_sb[d][:, T * m:T * (m + 1)], ps)
                            else:
                                nc.scalar.activation(
                                    xwt_sb[d][:, T * m:T * (m + 1)], ps, AF.Copy)

            def scan_phase(layer):
                # gate layout (permuted on host): cols 0:4=f, 4:8=i, 8:12=g~,
                # 12:16=o with g~ pre-activations DOUBLED (host scaled the
                # weights), so sigmoid over the g~ cols gives
                # tanh(x) = 2*sigmoid(2x) - 1.
                # The 16 gate columns accumulate as THREE psum groups
                # (f | i,g~ | o) so the pointwise chain starts while the
                # same direction's remaining matvecs are still on the PE;
                # only sigmoid(o) + the final h-mul stay exposed, and those
                # hide under the other direction's matvecs.
                GROUPS = ((0, 4), (4, 12), (12, 16))
                with tc.tile_pool(name=f"psg{layer}", bufs=4, space="PSUM") as p_g, \
                     tc.tile_pool(name=f"sg{layer}", bufs=6) as sgp:
                    for s in range(n_steps):
                        g_st = [None, None]
                        S_st = [None, None]
                        # all matvec groups (both dirs) first: the zero-time
                        # scheduler bakes its virtual order into sem waits,
                        # so pointwise ops must not precede matvecs there
                        for d in (0, 1):
                            t = s if d == 0 else T - 1 - s
                            hh = hh_sb[layer][d]
                            if s == 0:
                                h_prev = h0_sb[layer][d]
                            else:
                                tp = t - 1 if d == 0 else t + 1
                                h_prev = hh[:, 4 * tp:4 * tp + 4]
                            g_all = p_g.tile([128, 16], F32, name=f"g{d}", tag=f"g{d}")
                            g_st[d] = g_all
                            xwr = xwt_sb[d][:].rearrange("p (m t) -> p m t", t=T)
                            # ONE seed for all 16 cols (saves two 128-col
                            # ident LDWEIGHTS per dir-step); the per-group
                            # stop flags + the split sigmoid READS are what
                            # give the group pipelining, not split seeds
                            nc.tensor.matmul(g_all, ident_sb, xwr[:, :, t],
                                             start=True, stop=False,
                                             skip_group_check=True)
                            for m0, m1 in GROUPS:
                                for m in range(m0, m1):
                                    for kc in range(4):
                                        nc.tensor.matmul(
                                            g_all[:, m:m + 1],
                                            whh_sb[layer][d][kc][:, 128 * m:128 * (m + 1)],
                                            h_prev[:, kc:kc + 1],
                                            start=False,
                                            stop=(m == m1 - 1 and kc == 3),
                                            skip_group_check=True)
                        # pointwise chains, chained per engine to pin order
                        # (all TT ops on DVE: the Pool engine's ~170ns/op
                        # dispatch latency makes a split-engine variant lose)
                        for d in (0, 1):
                            t = s if d == 0 else T - 1 - s
                            ve = nc.vector
                            vkey = "vec"
                            hh = hh_sb[layer][d]
                            g_all = g_st[d]
                            S = sgp.tile([128, 16], F32, name=f"S{d}", tag=f"S{d}")
                            cc = c_sb[layer][d]
                            t1 = sgp.tile([128, 4], F32, name=f"t1{d}", tag=f"t1{d}")
                            u2 = sgp.tile([128, 4], F32, name=f"u2{d}", tag=f"u2{d}")
                            tct = sgp.tile([128, 4], F32, name=f"tc{d}", tag=f"tc{d}")
                            i1 = nc.scalar.activation(S[:, 0:4], g_all[:, 0:4],
                                                      AF.Sigmoid)
                            tc.chain_iter_dep("scal", i1.ins)
                            i2 = ve.tensor_mul(t1, S[:, 0:4], cc)
                            tc.chain_iter_dep(vkey, i2.ins)
                            i1 = nc.scalar.activation(S[:, 4:8], g_all[:, 4:8],
                                                      AF.Sigmoid)
                            tc.chain_iter_dep("scal", i1.ins)
                            i1 = nc.scalar.activation(S[:, 8:12], g_all[:, 8:12],
                                                      AF.Tanh)
                            tc.chain_iter_dep("scal", i1.ins)
                            # c' = sig(f)*c + sig(i)*tanh(g~)
                            i2 = ve.tensor_mul(u2, S[:, 4:8], S[:, 8:12])
                            tc.chain_iter_dep(vkey, i2.ins)
                            i3 = ve.tensor_add(cc, t1, u2)
                            tc.chain_iter_dep(vkey, i3.ins)
                            i1 = nc.scalar.activation(S[:, 12:16],
                                                      g_all[:, 12:16],
                                                      AF.Sigmoid)
                            tc.chain_iter_dep("scal", i1.ins)
                            i2 = nc.scalar.activation(tct, cc, AF.Tanh)
                            tc.chain_iter_dep("scal", i2.ins)
                            i3 = ve.tensor_mul(hh[:, 4 * t:4 * t + 4],
                                               S[:, 12:16], tct)
                            tc.chain_iter_dep(vkey, i3.ins)

            xwt_phase(0)
            # prefetch layer-1 weights + scoring constants during the L0
            # scan (DMAs have no deps on the scan, so they overlap with it)
            wih1_pre = [[], []]
            with tc.tile_pool(name="wihpre1", bufs=1) as wpre:
                for d in (0, 1):
                    for kc in range(8):
                        w = wpre.tile([128, G], BF, name=f"wihpre{d}{kc}")
                        nc.sync.dma_start(w, d_wihT1[d][128 * kc:128 * (kc + 1), :])
                        wih1_pre[d].append(w)
                w2rep_sb = load_deferred_consts()
                scan_phase(0)
                # ones row for layer-1 bias trick: x1 dim 416 -> (c=3, p=32) of
                # fwd hist (DVE start partition must be 32-aligned; 416 is pad)
                hh0f_r = hh_sb[0][0][:].rearrange("p (t c) -> p c t", c=4)
                nc.vector.memset(hh0f_r[32:33, 3, :], 1.0)
                xwt_phase(1, preloaded=wih1_pre)
            scan_phase(1)
            hh1f_r = hh_sb[1][0][:].rearrange("p (t c) -> p c t", c=4)
            nc.vector.memset(hh1f_r[32:33, 3, :], 1.0)

            # ---------- pairwise scoring ----------
            def hvecT_chunk(kc, jt):
                """lhsT [128, 128]: hvec.T rows [128kc:128kc+128], cols [128jt:+128]."""
                hhr = hh_sb[1][kc // 4][:].rearrange("p (t c) -> p c t", c=4)
                return hhr[:, kc % 4, 128 * jt:128 * (jt + 1)]

            with tc.tile_pool(name="pw", bufs=1) as pw:
                pj_sb, pi_sb = [], []
                with tc.tile_pool(name="pwps", bufs=2, space="PSUM") as pwps:
                    for jt in range(2):
                        ps = pwps.tile([128, MLP], F32, name=f"pjps{jt}", tag="projps")
                        for kc in range(8):
                            nc.tensor.matmul(ps, hvecT_chunk(kc, jt), w1bT_sb[kc],
                                             start=(kc == 0), stop=(kc == 7))
                        pj = pw.tile([128, MLP], BF, name=f"pj{jt}")
                        nc.vector.tensor_copy(pj, ps)
                        pj_sb.append(pj)
                    for jt in range(2):
                        ps = pwps.tile([128, MLP], F32, name=f"pips{jt}", tag="projps")
                        for kc in range(8):
                            nc.tensor.matmul(ps, hvecT_chunk(kc, jt), w1aT_sb[kc],
                                             start=(kc == 0), stop=(kc == 7))
                        pi = pw.tile([128, MLP], BF, name=f"pi{jt}")
                        nc.vector.tensor_copy(pi, ps)
                        pi_sb.append(pi)
                    # select this core's 32 head rows: pik = skT.T @ pi  [32, 400]
                    ps = pwps.tile([RPC, MLP], F32, name="pikps", tag="projps")
                    for kc in range(2):
                        nc.tensor.matmul(ps, skT_sb[kc], pi_sb[kc],
                                         start=(kc == 0), stop=(kc == 1))
                    pik = pw.tile([RPC, MLP], BF, name="pik")
                    nc.vector.tensor_copy(pik, ps)
                pik_flat = pw.tile([1, RPC * MLP], BF, name="pikflat")
                nc.sync.dma_start(
                    pik_flat[:].rearrange("p (a b) -> p a b", a=RPC),
                    pik)
                scoresT = [pw.tile([128, RPC], F32, name=f"scoresT{jc}")
                           for jc in range(2)]

                with tc.tile_pool(name="bps", bufs=2, space="PSUM") as bps, \
                     tc.tile_pool(name="bsb", bufs=3) as bsb:
                    for ig in range(RPC // 4):
                        for jc in range(2):
                            # 512-strided slots: matmul out must stay in 1 bank
                            B_ps = bps.tile([128, 4 * 512], F32, name=f"bps{ig}{jc}",
                                            tag="bps")
                            for l in range(4):
                                r = 4 * ig + l
                                # row-broadcast of pik row r, then pj added
                                # in-psum via identity matmul (keeps the
                                # 64 adds off the DVE, which binds scoring)
                                nc.tensor.matmul(
                                    B_ps[:, 512 * l:512 * l + MLP], ones_sb,
                                    pik_flat[:, MLP * r:MLP * (r + 1)],
                                    start=True, stop=False)
                                nc.tensor.matmul(
                                    B_ps[:, 512 * l:512 * l + MLP], ident_sb,
                                    pj_sb[jc], start=False, stop=True)
                            Tact = bsb.tile([128, 4 * MLP], BF, name=f"tact{ig}{jc}",
                                            tag="tact")
                            for l in range(4):
                                nc.scalar.activation(
                                    Tact[:, MLP * l:MLP * (l + 1)],
                                    B_ps[:, 512 * l:512 * l + MLP], AF.Tanh)
                            scr = bsb.tile([128, 4 * MLP], BF, name=f"scr{ig}{jc}",
                                           tag="scr")
                            for l in range(4):
                                me = nc.vector if l % 2 == 0 else nc.gpsimd
                                me.tensor_mul(
                                    scr[:, MLP * l:MLP * (l + 1)],
                                    Tact[:, MLP * l:MLP * (l + 1)], w2rep_sb)
                            nc.vector.tensor_reduce(
                                scoresT[jc][:, 4 * ig:4 * ig + 4].rearrange(
                                    "p (a b) -> p a b", b=1),
                                scr[:].rearrange("p (a b) -> p a b", a=4),
                                axis=mybir.AxisListType.X, op=ALU.add)
                for jc in range(2):
                    nc.sync.dma_start(d_out[128 * jc:128 * (jc + 1), :], scoresT[jc])

    nc.compile()
    _PROG_CACHE[key] = nc
    return nc


def _try_hw(n_steps, cores=1):
    """Debug helper: run a reduced-step program on HW with random inputs."""
    import jax
    from concourse import bass_utils
    from concourse.bass_interp import get_hw_module
    cpu = jax.devices("cpu")[0]
    rng = np.random.default_rng(0)
    fake = {
        "words": rng.integers(0, 50000, (T,)), "tags": rng.integers(0, 50, (T,)),
        "word_emb": rng.standard_normal((50000, 300), np.float32),
        "tag_emb": rng.standard_normal((50, 100), np.float32),
        "Wih0": rng.standard_normal((2, 1600, 400), np.float32) * 0.05,
        "Whh0": rng.standard_normal((2, 1600, 400), np.float32) * 0.05,
        "bih0": rng.standard_normal((2, 1600), np.float32) * 0.05,
        "bhh0": rng.standard_normal((2, 1600), np.float32) * 0.05,
        "Wih1": rng.standard_normal((2, 1600, 800), np.float32) * 0.05,
        "Whh1": rng.standard_normal((2, 1600, 400), np.float32) * 0.05,
        "bih1": rng.standard_normal((2, 1600), np.float32) * 0.05,
        "bhh1": rng.standard_normal((2, 1600), np.float32) * 0.05,
        "W1": rng.standard_normal((400, 1600), np.float32) * 0.05,
        "b1": rng.standard_normal((400,), np.float32) * 0.05,
        "W2": rng.standard_normal((1, 400), np.float32) * 0.05,
        "b2": rng.standard_normal((1,), np.float32) * 0.05,
        "h0": rng.standard_normal((2, 2, 400), np.float32),
        "c0": rng.standard_normal((2, 2, 400), np.float32),
    }
    nc = _get_program(n_steps)
    in_maps, I = _prep_inputs(fake)
    old = nc.m
    nc.m = get_hw_module(nc.m)
    try:
        res = bass_utils.run_bass_kernel_spmd(nc, in_maps[:cores],
                                              core_ids=list(range(cores)))
    finally:
        nc.m = old
    print(f"n_steps={n_steps} cores={cores}: OK,",
          res.results[0]["scores_t"].shape)


def _prep_inputs(inputs):
    """Host-side prep: gather embeddings, build padded/permuted device tensors."""
    I = {k: np.asarray(v) for k, v in inputs.items()}
    x = np.concatenate([I["word_emb"][I["words"]], I["tag_emb"][I["tags"]]],
                       axis=1).astype(np.float32)          # [T, 400]
    xT = np.zeros((HP, T), np.float32)
    xT[:H] = x.T
    xT[H] = 1.0                                            # bias row

    common = {"xT": xT.astype(BF16)}
    for l in (0, 1):
        Din = H if l == 0 else 2 * H
        DinP = HP if l == 0 else D1P
        for di, d in enumerate("fb"):
            wih = _permute_pad_gate_rows(I[f"Wih{l}"][di])  # [2048, Din]
            if l == 0:
                wihp = _pad_cols(wih, HP)                   # [2048, 512]
            else:
                wihp = np.zeros((G, D1P), np.float32)
                wihp[:, :H] = wih[:, :H]                    # fwd part
                wihp[:, HP:HP + H] = wih[:, H:2 * H]        # bwd part
            bias = _permute_pad_gate_vec(I[f"bih{l}"][di] + I[f"bhh{l}"][di])
            # bias column: layer 0's ones-row is xT row 400; layer 1's is the
            # hist pad position 416 (partition-32-aligned for the memset)
            wihp[:, H if l == 0 else 416] += bias
            common[f"wihT{l}{d}"] = np.ascontiguousarray(wihp.T).astype(BF16)

            whh = _pad_cols(_permute_pad_gate_rows(I[f"Whh{l}"][di]), HP)
            common[f"whhT{l}{d}"] = np.ascontiguousarray(whh.T).astype(BF16)

            common[f"h0_{l}{d}"] = _h_tile(I["h0"][l, di]).astype(BF16)
            common[f"c0_{l}{d}"] = _h_tile(I["c0"][l, di]).astype(np.float32)

    W1 = I["W1"].astype(np.float32)                         # [400, 1600]
    W1a, W1b = W1[:, :2 * H], W1[:, 2 * H:]                 # [400, 800] each

    def mlp_T(W, bias=None):
        Wp = np.zeros((MLP, D1P), np.float32)
        Wp[:, :H] = W[:, :H]
        Wp[:, HP:HP + H] = W[:, H:]
        if bias is not None:
            Wp[:, 416] += bias                              # hvec ones-row at 416
        return np.ascontiguousarray(Wp.T).astype(BF16)      # [1024, 400]

    common["w1aT"] = mlp_T(W1a, I["b1"].astype(np.float32))
    common["w1bT"] = mlp_T(W1b)
    common["w2rep"] = np.broadcast_to(I["W2"][0].astype(np.float32),
                                      (128, MLP)).astype(BF16).copy()
    common["ident"] = np.eye(128, dtype=np.float32).astype(BF16)

    in_maps = []
    for k in range(NCORES):
        m = dict(common)
        sk = np.zeros((T, RPC), np.float32)
        sk[RPC * k + np.arange(RPC), np.arange(RPC)] = 1.0
        m["skT"] = sk.astype(BF16)
        in_maps.append(m)
    return in_maps, I


def _ensure_ntff_hook():
    """Shim antenv.axon_hooks (absent in this image) so trace=True works."""
    import sys
    import types
    import antenv
    if hasattr(antenv, "axon_hooks") or "antenv.axon_hooks" in sys.modules:
        return
    hook = None
    try:
        from trn_agent_boot.trn_boot import _ntff_profile_via_ctypes
        hook = _ntff_profile_via_ctypes("/opt/axon/libaxon_pjrt.so")
    except Exception:
        hook = None
    mod = types.ModuleType("antenv.axon_hooks")
    state = {"hook": hook}
    mod.get_axon_ntff_profile_hook = lambda: state["hook"]
    mod.set_axon_ntff_profile_hook = lambda h: state.update(hook=h)
    sys.modules["antenv.axon_hooks"] = mod
    antenv.axon_hooks = mod


def kernel(**inputs):
    from concourse import bass_utils
    from concourse.bass_interp import get_hw_module

    nc = _get_program()
    in_maps, I = _prep_inputs(inputs)

    trace = bool(int(os.environ.get("KERNEL_TRACE", "0")))
    if trace:
        _ensure_ntff_hook()
    old_m = nc.m
    nc.m = get_hw_module(nc.m)
    try:
        res = bass_utils.run_bass_kernel_spmd(
            nc, in_maps, core_ids=list(range(NCORES)), trace=trace)
    finally:
        nc.m = old_m
    if trace and res.exec_time_ns is not None:
        print(f"HW exec time: {res.exec_time_ns} ns")
        kernel.last_exec_time_ns = res.exec_time_ns

    scores = np.zeros((T, T), np.float32)
    for k in range(NCORES):
        scores[RPC * k:RPC * (k + 1), :] = res.results[k]["scores_t"].T
    scores += float(I["b2"][0])
    scores[np.arange(T), np.arange(T)] = 0.0
    return scores



# revision 2
# speedup vs baseline: 1.1435x; 1.1435x over previous
"""Trainium2 Bass kernel for the DependencyParseModel problem — direction-split.

Sharding: 8 cores; even cores run the FORWARD direction of both biLSTM
layers, odd cores the BACKWARD direction, each on a 256-step scan with only
64 recurrent matvecs + 1 seed per step (vs 130 when both directions share a
core).  The direction is encoded purely in per-core input data (bwd cores
get time-reversed embeddings and their direction's weights), so the SPMD
program is identical on all cores:

  1. host: embedding gather, per-core weight permute/pad (direction select,
     layer-1 / scoring weight halves ordered [mine|peer], time reversal)
  2. device: XW0, L0 scan, pair AllGather to fetch the opposite direction's
     hidden history (peer recovered slot-symmetrically as s0+s1-mine, then
     time-flipped with a strided SBUF->SBUF DMA), XW1, L1 scan, second
     exchange, pairwise-MLP scoring for this core's 32 head rows
  3. host: un-reverse odd cores' outputs, assemble [256,256], add b2,
     zero diagonal

Pointwise gate order [g~, f, i, o] keeps only sigmoid(o) -> tanh(c') -> h
exposed after the last matvec group of each step.

Self-contained: hardcodes all shapes; no sibling imports.
"""

import os
import numpy as np
import ml_dtypes

T = 256
H = 400            # LSTM hidden
HP = 512           # padded hidden
G = 2048           # padded gate dim (4 gates x 512)
D1P = 1024         # padded layer-1 input dim (2 x HP)
MLP = 400
MLPP = 512         # padded MLP dim (scoring)
NCORES = 8
RPC = T // NCORES  # rows per core (head-word rows)
HHC = 4 * T        # hh tile cols

BF16 = ml_dtypes.bfloat16

# gate blocks in layout order [g~, f, i, o]; source ranges in torch order
# (i, f, g, o).  g~ first so tanh(g~) pipelines early; o last so only
# sigmoid(o) + tanh(c') + h-mul trail the final matvec group.
_GATE_SRC = [(800, 1200), (400, 800), (0, 400), (1200, 1600)]


def _permute_pad_gate_rows(W):
    """[1600, K] -> [2048, K]: torch gate order i,f,g,o -> blocks [g~,f,i,o]."""
    out = np.zeros((G, W.shape[1]), np.float32)
    for b, (s, e) in enumerate(_GATE_SRC):
        out[b * 512: b * 512 + (e - s)] = W[s:e]
    return out


def _permute_pad_gate_vec(v):
    out = np.zeros(G, np.float32)
    for b, (s, e) in enumerate(_GATE_SRC):
        out[b * 512: b * 512 + (e - s)] = v[s:e]
    return out


def _pad_cols(W, K):
    out = np.zeros((W.shape[0], K), np.float32)
    out[:, : W.shape[1]] = W
    return out


def _h_tile(v):
    """[400] -> [128, 4] tile, elem d -> (d % 128, d // 128)."""
    out = np.zeros(HP, np.float32)
    out[:H] = v
    return np.ascontiguousarray(out.reshape(4, 128).T)


_PROG_CACHE = {}


def _get_program(n_steps=T, ncores=NCORES):
    key = ("prog", n_steps, ncores)
    if key in _PROG_CACHE:
        return _PROG_CACHE[key]

    import concourse.bass as bass
    import concourse.mybir as mybir
    import concourse.tile as tile
    from concourse import bacc
    from concourse import hw_specs as _hs

    # Price the PE decode at the real ~27ns/matvec FWL pace for any
    # consumer of hw_specs (CoreSim etc).
    _hs.TRN2Spec.EXPECTED_HWDECODE_OVERHEAD_NS[mybir.EngineType.PE] = 27.0

    F32 = mybir.dt.float32
    BF = mybir.dt.bfloat16
    AF = mybir.ActivationFunctionType
    ALU = mybir.AluOpType

    nc = bacc.Bacc("TRN2", target_bir_lowering=False, debug=False,
                   enable_asserts=False, num_devices=ncores)
    groups = [[2 * g, 2 * g + 1] for g in range(ncores // 2)]

    # ---- DRAM I/O (single direction per core) ----
    d_xT = nc.dram_tensor("xT", [HP, T], BF, kind="ExternalInput").ap()
    d_wihT0 = nc.dram_tensor("wihT0", [HP, G], BF, kind="ExternalInput").ap()
    d_wihT1 = nc.dram_tensor("wihT1", [D1P, G], BF, kind="ExternalInput").ap()
    d_whhT = [nc.dram_tensor(f"whhT{l}", [HP, G], BF, kind="ExternalInput").ap()
              for l in (0, 1)]
    d_h0 = [nc.dram_tensor(f"h0_{l}", [128, 4], BF, kind="ExternalInput").ap()
            for l in (0, 1)]
    d_c0 = [nc.dram_tensor(f"c0_{l}", [128, 4], F32, kind="ExternalInput").ap()
            for l in (0, 1)]
    d_w1aT = nc.dram_tensor("w1aT", [D1P, MLPP], BF, kind="ExternalInput").ap()
    d_w1bT = nc.dram_tensor("w1bT", [D1P, MLPP], BF, kind="ExternalInput").ap()
    d_skT = nc.dram_tensor("skT", [T, RPC], BF, kind="ExternalInput").ap()
    d_w2colT = nc.dram_tensor("w2colT", [128, 4], BF, kind="ExternalInput").ap()
    d_ident = nc.dram_tensor("ident", [128, 128], BF, kind="ExternalInput").ap()
    d_out = nc.dram_tensor("scores_r", [RPC, T], F32, kind="ExternalOutput").ap()

    with tile.TileContext(nc) as tc:
        from contextlib import ExitStack
        with ExitStack() as ctx:
            const = ctx.enter_context(tc.tile_pool(name="const", bufs=1))
            state = ctx.enter_context(tc.tile_pool(name="state", bufs=1))
            whhp = ctx.enter_context(tc.tile_pool(name="whhp", bufs=1))
            dram = ctx.enter_context(tc.tile_pool(name="dram", bufs=1, space="DRAM"))

            # --- constants / initial state in SBUF (DMA queue drains in
            # emission order: XW0 + L0-scan needs first) ---
            xT_sb = []
            for kc in range(4):
                xt = const.tile([128, T], BF, name=f"xT{kc}")
                nc.sync.dma_start(xt, d_xT[128 * kc:128 * (kc + 1), :])
                xT_sb.append(xt)
            ident_sb = const.tile([128, 128], BF, name="ident")
            nc.sync.dma_start(ident_sb, d_ident)
            h0_sb = [None, None]
            c_sb = [None, None]
            for l in (0, 1):
                t0 = const.tile([128, 4], BF, name=f"h0sb{l}")
                nc.sync.dma_start(t0, d_h0[l])
                h0_sb[l] = t0
                t1 = state.tile([128, 4], F32, name=f"csb{l}")
                nc.sync.dma_start(t1, d_c0[l])
                c_sb[l] = t1
            whh_sb = [None, None]
            chunks = []
            for kc in range(4):
                w = whhp.tile([128, G], BF, name=f"whh0{kc}")
                nc.sync.dma_start(w, d_whhT[0][128 * kc:128 * (kc + 1), :])
                chunks.append(w)
            whh_sb[0] = chunks

            def load_deferred_consts():
                chunks = []
                for kc in range(4):
                    w = whhp.tile([128, G], BF, name=f"whh1{kc}")
                    nc.sync.dma_start(w, d_whhT[1][128 * kc:128 * (kc + 1), :])
                    chunks.append(w)
                whh_sb[1] = chunks
                for kc in range(8):
                    wa = const.tile([128, MLPP], BF, name=f"w1aT{kc}")
                    nc.sync.dma_start(wa, d_w1aT[128 * kc:128 * (kc + 1), :])
                    w1aT_sb.append(wa)
                    wb = const.tile([128, MLPP], BF, name=f"w1bT{kc}")
                    nc.sync.dma_start(wb, d_w1bT[128 * kc:128 * (kc + 1), :])
                    w1bT_sb.append(wb)
                for kc in range(2):
                    sk = const.tile([128, RPC], BF, name=f"skT{kc}")
                    nc.sync.dma_start(sk, d_skT[128 * kc:128 * (kc + 1), :])
                    skT_sb.append(sk)
                w2 = const.tile([128, 4], BF, name="w2colT")
                nc.sync.dma_start(w2, d_w2colT)
                return w2

            w1aT_sb, w1bT_sb, skT_sb = [], [], []

            # persistent per-layer hidden history [128, 4*T] bf16,
            # col 4s+c = h_s[128c+p] (s = my-order step index); plus the
            # peer direction's history, already flipped into my-order.
            hh_sb = [state.tile([128, HHC], BF, name=f"hh{l}") for l in (0, 1)]
            peer_sb = [state.tile([128, HHC], BF, name=f"peer{l}") for l in (0, 1)]
            # XW^T buffer: [128, 16*T] bf16, col m*T + t
            xwt_sb = state.tile([128, 16 * T], BF, name="xwt")
            # exchange scratch
            exs0 = state.tile([128, HHC], BF, name="exs0")
            exs1 = state.tile([128, HHC], BF, name="exs1")
            exq = state.tile([128, HHC], F32, name="exq")
            exp_ = state.tile([128, HHC], BF, name="exp")

            def exchange(layer):
                """peer_sb[layer] <- other pair-core's hh[layer], time-flipped.

                Slot-symmetric: peer = (slot0 + slot1) - mine, all in fp32
                (exact for bf16 inputs), so the same program works for both
                pair members.
                """
                hh = hh_sb[layer]
                cc_in = dram.tile([128, HHC], BF, name=f"ccin{layer}")
                cc_out = dram.tile([2, 128, HHC], BF, name=f"ccout{layer}")
                nc.sync.dma_start(cc_in[:], hh)
                nc.gpsimd.collective_compute(
                    "AllGather", mybir.AluOpType.bypass,
                    replica_groups=groups,
                    ins=[cc_in[:]], outs=[cc_out[:]])
                nc.sync.dma_start(exs0, cc_out[0])
                nc.sync.dma_start(exs1, cc_out[1])
                nc.vector.tensor_tensor(exq, exs0, exs1, op=ALU.add)
                nc.vector.tensor_tensor(exp_, exq, hh, op=ALU.subtract)
                # time-flip into my-order via strided DMA copy
                nc.sync.dma_start(
                    peer_sb[layer][:].rearrange("p (t c) -> p t c", c=4),
                    exp_[:].rearrange("p (t c) -> p t c", c=4)[:, ::-1, :])

            def xwt_phase(layer, preloaded=None):
                """xwt_sb <- Wih[layer] @ x_aug (all timesteps, my dir)."""
                K = 4 if layer == 0 else 8
                d_wih = d_wihT0 if layer == 0 else d_wihT1
                with tc.tile_pool(name=f"wihp{layer}", bufs=1) as wp, \
                     tc.tile_pool(name=f"xwps{layer}", bufs=8, space="PSUM") as pp:
                    if preloaded is not None:
                        wih_sb = preloaded
                    else:
                        wih_sb = []
                        for kc in range(K):
                            w = wp.tile([128, G], BF, name=f"wih{layer}{kc}",
                                        tag=f"wih{kc}")
                            nc.sync.dma_start(w, d_wih[128 * kc:128 * (kc + 1), :])
                            wih_sb.append(w)
                    if layer == 0:
                        rhs = xT_sb
                    else:
                        rhs = []
                        for kc in range(8):
                            src = hh_sb[0] if kc < 4 else peer_sb[0]
                            hhr = src[:].rearrange("p (t c) -> p c t", c=4)
                            rhs.append(hhr[:, kc % 4, :])
                    for m in range(16):
                        ps = pp.tile([128, T], F32, name=f"xwps{layer}{m}",
                                     tag="xwps")
                        for kc in range(K):
                            nc.tensor.matmul(
                                ps, wih_sb[kc][:, 128 * m:128 * (m + 1)], rhs[kc],
                                start=(kc == 0), stop=(kc == K - 1))
                        if m % 2 == 0:
                            nc.vector.tensor_copy(
                                xwt_sb[:, T * m:T * (m + 1)], ps)
                        else:
                            nc.scalar.activation(
                                xwt_sb[:, T * m:T * (m + 1)], ps, AF.Copy)

            def scan_phase(layer):
                # gates in matvec order [g~, f, i, o], ONE FULL PSUM BANK per
                # gate (accumulation-group close is tracked per bank: sharing
                # a bank serializes every gate's activation behind the whole
                # step's matvecs).  4 gates x bufs=2 x 2KB = all 8 banks.
                #
                # Emission is software-pipelined one step ahead: step s+1's
                # seed matmuls are emitted right after step s's matvecs, so
                # the zero-time scheduler doesn't thread their sem waits
                # through step s's pointwise (which would serialize sigma(o)
                # behind them).  Matvecs go kc-major within a gate and the
                # h write is split in column halves, so the next step's
                # first matvecs start off the first half of h.
                hh = hh_sb[layer]
                xwr = xwt_sb[:].rearrange("p (m t) -> p m t", t=T)
                with tc.tile_pool(name=f"psg{layer}", bufs=1, space="PSUM") as p_g, \
                     tc.tile_pool(name=f"sg{layer}", bufs=6) as sgp:

                    def emit_seeds(s):
                        # tag alternates by step parity: same-tag psum allocs
                        # serialize write-after-read one-deep, so parity tags
                        # give two-step pipelining (8 tags x 2KB banks = PSUM)
                        g_t = []
                        for gi in range(4):
                            gt = p_g.tile([128, 512], F32, name=f"g{gi}",
                                          tag=f"g{gi}{s % 2}")
                            g_t.append(gt)
                            nc.tensor.matmul(gt[:, 0:4], ident_sb,
                                             xwr[:, 4 * gi:4 * gi + 4, s],
                                             start=True, stop=False,
                                             skip_group_check=True)
                        return g_t

                    def emit_matvecs(s, g_t):
                        if s == 0:
                            h_prev = h0_sb[layer]
                        else:
                            h_prev = hh[:, 4 * (s - 1):4 * (s - 1) + 4]
                        for gi in range(4):
                            for kc in range(4):
                                for ml in range(4):
                                    m = 4 * gi + ml
                                    nc.tensor.matmul(
                                        g_t[gi][:, ml:ml + 1],
                                        whh_sb[layer][kc][:, 128 * m:128 * (m + 1)],
                                        h_prev[:, kc:kc + 1],
                                        start=False,
                                        stop=(kc == 3 and ml == 3),
                                        skip_group_check=True)

                    def emit_pointwise(s, g_t):
                        S = sgp.tile([128, 16], F32, name="S", tag="S")
                        cc = c_sb[layer]
                        t1 = sgp.tile([128, 4], F32, name="t1", tag="t1")
                        u2 = sgp.tile([128, 4], F32, name="u2", tag="u2")
                        tct = sgp.tile([128, 4], F32, name="tc", tag="tc")
                        i1 = nc.scalar.activation(S[:, 0:4], g_t[0][:, 0:4],
                                                  AF.Tanh)
                        tc.chain_iter_dep("scal", i1.ins)
                        i1 = nc.scalar.activation(S[:, 4:8], g_t[1][:, 0:4],
                                                  AF.Sigmoid)
                        tc.chain_iter_dep("scal", i1.ins)
                        i2 = nc.vector.tensor_mul(t1, S[:, 4:8], cc)
                        tc.chain_iter_dep("vec", i2.ins)
                        i1 = nc.scalar.activation(S[:, 8:12], g_t[2][:, 0:4],
                                                  AF.Sigmoid)
                        tc.chain_iter_dep("scal", i1.ins)
                        i2 = nc.vector.tensor_mul(u2, S[:, 8:12], S[:, 0:4])
                        tc.chain_iter_dep("vec", i2.ins)
                        i3 = nc.vector.tensor_add(cc, t1, u2)
                        tc.chain_iter_dep("vec", i3.ins)
                        i1 = nc.scalar.activation(S[:, 12:16], g_t[3][:, 0:4],
                                                  AF.Sigmoid)
                        tc.chain_iter_dep("scal", i1.ins)
                        i2 = nc.scalar.activation(tct, cc, AF.Tanh)
                        tc.chain_iter_dep("scal", i2.ins)
                        # split the h write so the next step's kc<2 matvecs
                        # start as soon as the first half lands
                        i3 = nc.vector.tensor_mul(hh[:, 4 * s:4 * s + 2],
                                                  S[:, 12:14], tct[:, 0:2])
                        tc.chain_iter_dep("vec", i3.ins)
                        i3 = nc.vector.tensor_mul(hh[:, 4 * s + 2:4 * s + 4],
                                                  S[:, 14:16], tct[:, 2:4])
                        tc.chain_iter_dep("vec", i3.ins)

                    g_cur = emit_seeds(0)
                    for s in range(n_steps):
                        emit_matvecs(s, g_cur)
                        emit_pointwise(s, g_cur)
                        g_cur = emit_seeds(s + 1) if s + 1 < n_steps else None

            xwt_phase(0)
            # prefetch layer-1 weights + scoring constants during the L0 scan
            wih1_pre = []
            with tc.tile_pool(name="wihpre1", bufs=1) as wpre:
                for kc in range(8):
                    w = wpre.tile([128, G], BF, name=f"wihpre{kc}")
                    nc.sync.dma_start(w, d_wihT1[128 * kc:128 * (kc + 1), :])
                    wih1_pre.append(w)
                w2colT_sb = load_deferred_consts()
                scan_phase(0)
                # ones row for layer-1 bias trick: my-hist pad position 416
                hh0_r = hh_sb[0][:].rearrange("p (t c) -> p c t", c=4)
                nc.vector.memset(hh0_r[32:33, 3, :], 1.0)
                exchange(0)
                xwt_phase(1, preloaded=wih1_pre)
            scan_phase(1)
            hh1_r = hh_sb[1][:].rearrange("p (t c) -> p c t", c=4)
            nc.vector.memset(hh1_r[32:33, 3, :], 1.0)
            exchange(1)

            # ---------- pairwise scoring ----------
            # score[c, j] = sum_m w2[m] * tanh(pik[c, m] + pjT[m, j]); the
            # tanh runs on ACT with m on partitions and pik as per-partition
            # bias, and the w2 contraction is a [128,1].T @ [128,T] matmul.
            def hvecT_chunk(kc, jt=None):
                """lhsT [128, 128|256]: hvec.T rows [128kc:+128] (cols jt-block)."""
                src = hh_sb[1] if kc < 4 else peer_sb[1]
                hhr = src[:].rearrange("p (t c) -> p c t", c=4)
                if jt is None:
                    return hhr[:, kc % 4, :]
                return hhr[:, kc % 4, 128 * jt:128 * (jt + 1)]

            with tc.tile_pool(name="pw", bufs=1) as pw:
                pjT_sb = []
                pi_sb = []
                with tc.tile_pool(name="pwps", bufs=4, space="PSUM") as pwps:
                    # pjT[mb]: [128 m, 256 j] = sum_d w1bT[d, m-block].T @ hvecT[d, j]
                    for mb in range(4):
                        ps = pwps.tile([128, T], F32, name=f"pjTps{mb}",
                                       tag="projps")
                        for kc in range(8):
                            nc.tensor.matmul(
                                ps, w1bT_sb[kc][:, 128 * mb:128 * (mb + 1)],
                                hvecT_chunk(kc),
                                start=(kc == 0), stop=(kc == 7))
                        pjT = pw.tile([128, T], BF, name=f"pjT{mb}")
                        if mb % 2 == 0:
                            nc.vector.tensor_copy(pjT, ps)
                        else:
                            nc.scalar.activation(pjT, ps, AF.Copy)
                        pjT_sb.append(pjT)
                    # pi [128 j, 512 m] (two j-halves), then head-row select
                    for jt in range(2):
                        ps = pwps.tile([128, MLPP], F32, name=f"pips{jt}",
                                       tag="pips")
                        for kc in range(8):
                            nc.tensor.matmul(ps, hvecT_chunk(kc, jt), w1aT_sb[kc],
                                             start=(kc == 0), stop=(kc == 7))
                        pi = pw.tile([128, MLPP], BF, name=f"pi{jt}")
                        if jt == 0:
                            nc.vector.tensor_copy(pi, ps)
                        else:
                            nc.scalar.activation(pi, ps, AF.Copy)
                        pi_sb.append(pi)
                    # pik = skT.T @ pi  [32, 512]
                    ps = pwps.tile([RPC, MLPP], F32, name="pikps", tag="pips")
                    for kc in range(2):
                        nc.tensor.matmul(ps, skT_sb[kc], pi_sb[kc],
                                         start=(kc == 0), stop=(kc == 1))
                    pik = pw.tile([RPC, MLPP], BF, name="pik")
                    nc.vector.tensor_copy(pik, ps)
                    # pikT[mb]: [128 m, 32 c] via PE transpose
                    pikT_sb = []
                    for mb in range(4):
                        tps = pwps.tile([128, RPC], BF, name=f"pikTps{mb}",
                                        tag="projps")
                        nc.tensor.transpose(
                            tps, pik[:, 128 * mb:128 * (mb + 1)],
                            ident_sb[0:RPC, 0:RPC])
                        pikT = pw.tile([128, RPC], F32, name=f"pikT{mb}")
                        if mb % 2 == 0:
                            nc.vector.tensor_copy(pikT, tps)
                        else:
                            nc.scalar.activation(pikT, tps, AF.Copy)
                        pikT_sb.append(pikT)

                scores_flat = pw.tile([1, RPC * T], F32, name="scoresflat")
                with tc.tile_pool(name="scps", bufs=8, space="PSUM") as scps, \
                     tc.tile_pool(name="actp", bufs=6) as actp:
                    for c in range(RPC):
                        sc_ps = scps.tile([1, T], F32, name=f"scps{c}",
                                          tag="scps")
                        for mb in range(4):
                            actT = actp.tile([128, T], BF, name=f"act{c}{mb}",
                                             tag=f"act{mb % 2}")
                            nc.scalar.activation(actT, pjT_sb[mb], AF.Tanh,
                                                 bias=pikT_sb[mb][:, c:c + 1])
                            nc.tensor.matmul(sc_ps, w2colT_sb[:, mb:mb + 1],
                                             actT, start=(mb == 0),
                                             stop=(mb == 3))
                        nc.vector.tensor_copy(
                            scores_flat[:, T * c:T * (c + 1)], sc_ps)
                nc.sync.dma_start(
                    d_out,
                    scores_flat[:].rearrange("p (a b) -> p a b", a=RPC))

    nc.compile()
    _PROG_CACHE[key] = nc
    return nc


def _prep_inputs(inputs, ncores=NCORES):
    """Host-side prep: per-core direction select, reversal, half-swaps."""
    I = {k: np.asarray(v) for k, v in inputs.items()}
    x = np.concatenate([I["word_emb"][I["words"]], I["tag_emb"][I["tags"]]],
                       axis=1).astype(np.float32)          # [T, 400]
    xT_base = np.zeros((HP, T), np.float32)
    xT_base[:H] = x.T
    xT_base[H] = 1.0                                       # bias row

    W1 = I["W1"].astype(np.float32)                        # [400, 1600]
    W1a, W1b = W1[:, :2 * H], W1[:, 2 * H:]                # [400, 800] each

    def mlp_T(W, d, bias=None):
        """[400, 800] -> [1024, 512]: halves ordered [mine|peer] for dir d."""
        mine = W[:, :H] if d == 0 else W[:, H:]
        peer = W[:, H:] if d == 0 else W[:, :H]
        Wp = np.zeros((MLPP, D1P), np.float32)
        Wp[:MLP, :H] = mine
        Wp[:MLP, HP:HP + H] = peer
        if bias is not None:
            Wp[:MLP, 416] += bias                          # my-hist ones-row
        return np.ascontiguousarray(Wp.T).astype(BF16)

    per_dir = {}
    for d in (0, 1):
        m = {}
        m["xT"] = (xT_base if d == 0 else
                   np.ascontiguousarray(xT_base[:, ::-1])).astype(BF16)
        # layer 0
        wih = _permute_pad_gate_rows(I["Wih0"][d])         # [2048, 400]
        wihp = _pad_cols(wih, HP)
        bias = _permute_pad_gate_vec(I["bih0"][d] + I["bhh0"][d])
        wihp[:, H] += bias
        m["wihT0"] = np.ascontiguousarray(wihp.T).astype(BF16)
        whh = _pad_cols(_permute_pad_gate_rows(I["Whh0"][d]), HP)
        m["whhT0"] = np.ascontiguousarray(whh.T).astype(BF16)
        # layer 1: input halves [mine|peer]
        wih = _permute_pad_gate_rows(I["Wih1"][d])         # [2048, 800]
        mine_sl = slice(0, H) if d == 0 else slice(H, 2 * H)
        peer_sl = slice(H, 2 * H) if d == 0 else slice(0, H)
        wihp = np.zeros((G, D1P), np.float32)
        wihp[:, :H] = wih[:, mine_sl]
        wihp[:, HP:HP + H] = wih[:, peer_sl]
        bias = _permute_pad_gate_vec(I["bih1"][d] + I["bhh1"][d])
        wihp[:, 416] += bias
        m["wihT1"] = np.ascontiguousarray(wihp.T).astype(BF16)
        whh = _pad_cols(_permute_pad_gate_rows(I["Whh1"][d]), HP)
        m["whhT1"] = np.ascontiguousarray(whh.T).astype(BF16)
        for l in (0, 1):
            m[f"h0_{l}"] = _h_tile(I["h0"][l, d]).astype(BF16)
            m[f"c0_{l}"] = _h_tile(I["c0"][l, d]).astype(np.float32)
        m["w1aT"] = mlp_T(W1a, d, I["b1"].astype(np.float32))
        m["w1bT"] = mlp_T(W1b, d)
        per_dir[d] = m

    w2colT = np.zeros((128, 4), np.float32)
    w2flat = np.zeros(MLPP, np.float32)
    w2flat[:MLP] = I["W2"][0].astype(np.float32)
    w2colT[:, :] = w2flat.reshape(4, 128).T
    w2colT = w2colT.astype(BF16)
    ident = np.eye(128, dtype=np.float32).astype(BF16)

    in_maps = []
    for k in range(ncores):
        d = k % 2
        m = dict(per_dir[d])
        m["w2colT"] = w2colT
        m["ident"] = ident
        sk = np.zeros((T, RPC), np.float32)
        rows = RPC * k + np.arange(RPC)                    # true head rows
        a = rows if d == 0 else (T - 1) - rows             # my-order rows
        sk[a, np.arange(RPC)] = 1.0
        m["skT"] = sk.astype(BF16)
        in_maps.append(m)
    return in_maps, I


def _ensure_ntff_hook():
    """Shim antenv.axon_hooks (absent in this image) so trace=True works."""
    import sys
    import types
    import antenv
    if hasattr(antenv, "axon_hooks") or "antenv.axon_hooks" in sys.modules:
        return
    hook = None
    try:
        from trn_agent_boot.trn_boot import _ntff_profile_via_ctypes
        hook = _ntff_profile_via_ctypes("/opt/axon/libaxon_pjrt.so")
    except Exception:
        hook = None
    mod = types.ModuleType("antenv.axon_hooks")
    state = {"hook": hook}
    mod.get_axon_ntff_profile_hook = lambda: state["hook"]
    mod.set_axon_ntff_profile_hook = lambda h: state.update(hook=h)
    sys.modules["antenv.axon_hooks"] = mod
    antenv.axon_hooks = mod


def _assemble(results, I):
    """Per-core [32, 256] outputs -> full [256, 256] score matrix."""
    scores = np.zeros((T, T), np.float32)
    for k in range(NCORES):
        rows = results[k]["scores_r"]                      # [32, 256] my-order
        if k % 2 == 1:
            rows = rows[:, ::-1]                           # un-reverse j axis
        scores[RPC * k:RPC * (k + 1), :] = rows
    scores += float(I["b2"][0])
    scores[np.arange(T), np.arange(T)] = 0.0
    return scores


def kernel(**inputs):
    from concourse import bass_utils
    from concourse.bass_interp import get_hw_module

    nc = _get_program()
    in_maps, I = _prep_inputs(inputs)

    trace = bool(int(os.environ.get("KERNEL_TRACE", "0")))
    if trace:
        _ensure_ntff_hook()
    old_m = nc.m
    nc.m = get_hw_module(nc.m)
    try:
        res = bass_utils.run_bass_kernel_spmd(
            nc, in_maps, core_ids=list(range(NCORES)), trace=trace)
    finally:
        nc.m = old_m
    if trace and res.exec_time_ns is not None:
        print(f"HW exec time: {res.exec_time_ns} ns")
        kernel.last_exec_time_ns = res.exec_time_ns

    return _assemble(res.results, I)
